# revision 3
# baseline (speedup 1.0000x reference)
import sys, time
sys.path.insert(0, "/opt/trn_rl_repo")
import numpy as np
import ml_dtypes
from contextlib import ExitStack

import concourse.bass as bass
import concourse.tile as tile
import concourse.bass_isa as bass_isa
from concourse import mybir, bacc
from concourse.bass_utils import run_bass_kernel_spmd

BF16 = ml_dtypes.bfloat16
F8 = ml_dtypes.float8_e4m3
F32 = mybir.dt.float32
BF = mybir.dt.bfloat16
E4 = mybir.dt.float8e4
AF = mybir.ActivationFunctionType
OP = mybir.AluOpType
RED = bass_isa.ReduceOp

B, L, DM, ED, EDH, N, DT_RANK, NL = 4, 1024, 512, 1024, 512, 16, 32, 2
EPS = 1e-5
RG = [[0, 1], [2, 3], [4, 5], [6, 7]]

REPEAT = 1
LAST_RUN_S = 0.0
ABLATE = frozenset()
_CACHE = {}


def _build(repeat, kvals, has_cvb, mode=frozenset()):
    nc = bacc.Bacc("TRN2", target_bir_lowering=False, debug=False, num_devices=8)
    xT_d = nc.dram_tensor("xT", [128, 4096], BF, kind="ExternalInput")
    winT_d = nc.dram_tensor("winT", [128, 8192], E4, kind="ExternalInput")
    wout_d = nc.dram_tensor("wout", [128, 4096], E4, kind="ExternalInput")
    wxp_d = nc.dram_tensor("wxp", [128, 512], E4, kind="ExternalInput")
    wdt_d = nc.dram_tensor("wdt", [33, 1024], BF, kind="ExternalInput")
    cvw_d = nc.dram_tensor("cvw", [128, 32], F32, kind="ExternalInput")
    cvb_d = nc.dram_tensor("cvb", [128, 8], F32, kind="ExternalInput")
    Dv_d = nc.dram_tensor("Dv", [128, 8], F32, kind="ExternalInput")
    kvi_d = nc.dram_tensor("kvi", [128, 32], F32, kind="ExternalInput")
    fcp_d = nc.dram_tensor("fcp", [128, 4], F32, kind="ExternalInput")
    fcb_d = nc.dram_tensor("fcb", [1, 1], F32, kind="ExternalInput")
    out_d = nc.dram_tensor("out", [1, 1024], F32, kind="ExternalOutput")
    cc = {}
    for li in range(NL):
        # x_proj partials flat [1, 64*1024] so B/C rows can be re-read flat
        cc[("dbc_in", li)] = nc.dram_tensor(f"ccdbci{li}", [1, 65536], BF)
        cc[("dbc_out", li)] = nc.dram_tensor(f"ccdbco{li}", [1, 65536], BF)
        cc[("bo_in", li)] = nc.dram_tensor(f"ccboi{li}", [128, 4096], E4)
        cc[("bo_out", li)] = nc.dram_tensor(f"ccboo{li}", [128, 4096], E4)

    with tile.TileContext(nc) as tc, ExitStack() as ctx:
        sb = ctx.enter_context(tc.tile_pool(name="sb", bufs=1))
        pp = ctx.enter_context(
            tc.tile_pool(name="pp", bufs=1, space=bass.MemorySpace.PSUM))

        # ---- persistent weights ----
        winT = sb.tile([128, 8192], E4)
        nc.sync.dma_start(winT[:], winT_d[:])
        wout = sb.tile([128, 4096], E4)
        nc.sync.dma_start(wout[:], wout_d[:])
        wxp = sb.tile([128, 512], E4)
        nc.sync.dma_start(wxp[:], wxp_d[:])
        wdt = sb.tile([33, 1024], BF)
        nc.sync.dma_start(wdt[:], wdt_d[:])
        cvw = sb.tile([128, 32], F32)
        nc.sync.dma_start(cvw[:], cvw_d[:])
        cvb = sb.tile([128, 8], F32)
        nc.sync.dma_start(cvb[:], cvb_d[:])
        Dv = sb.tile([128, 8], F32)
        nc.sync.dma_start(Dv[:], Dv_d[:])
        kvi = sb.tile([128, 32], F32)
        nc.sync.dma_start(kvi[:], kvi_d[:])
        fcp = sb.tile([128, 4], F32)
        nc.sync.dma_start(fcp[:], fcp_d[:])
        fcb = sb.tile([1, 1], F32)
        nc.sync.dma_start(fcb[:], fcb_d[:])
        epsc = sb.tile([128, 1], F32)
        nc.vector.memset(epsc[:], EPS)

        # ---- dynamic tiles ----
        xT = sb.tile([128, 4096], BF)        # residual stream, [dsub, dc*1024+t]
        xn = sb.tile([128, 4096], BF)        # conv acc/xin -> y3
        xq = sb.tile([128, 4096], E4)        # fp8 rms-x for in_proj
        xinq = sb.tile([128, 4096], E4)      # fp8 xin for x_proj
        y3q = sb.tile([128, 4096], E4)       # fp8 gate output for out_proj
        xinpad = sb.tile([128, 4108], BF)    # 4 blocks of [3 pad | 1024]; sz overlays [:, 0:4096]
        lu = sb.tile([128, 4096], BF)        # delta -> bo
        ut = sb.tile([128, 4096], BF)        # u = delta * xin
        dbcdt = sb.tile([33, 1024], BF)      # dt-rank rows of dbc + ones row
        BCmat = sb.tile([128, 32768], BF)
        dA = sb.tile([128, 16384], BF)
        dBxh = sb.tile([128, 16384], BF)
        dBf = dBxh[:].bitcast(F32)           # [128, 8192] f32 scratch views

        nc.vector.memset(dbcdt[32:33, :], 1.0)
        # zero dA n-block boundary columns once; scan ops never write them
        nc.vector.memset(dA[:].rearrange("p (n l) -> p n l", n=N)[:, :, 0:1], 0.0)

        MM = nc.tensor.matmul
        ACT = nc.scalar.activation
        TT = nc.vector.tensor_tensor
        TSM = nc.vector.tensor_scalar_mul

        def nlv(t):
            return t[:].rearrange("p (n l) -> p n l", n=N)

        def mtv(ap):
            return ap.rearrange("p (m t) -> p m t", m=4)

        def bc_n(ap2d):
            # [128, 1024] -> [128, N, 1024] stride-0 n dim
            return ap2d.unsqueeze(1).broadcast_to([128, N, 1024])

        def bc_t(ap2d, n=1024):
            # [128, m] -> [128, m, n] stride-0 inner dim
            return ap2d.unsqueeze(2).broadcast_to([128, ap2d.shape[1], n])

        for _r in range(repeat):
            nc.sync.dma_start(xT[:], xT_d[:])
            for li in range(NL):
                # ================= rmsnorm =================
                TT(dBf[:, 0:4096], xT[:], xT[:], OP.mult)
                nc.vector.tensor_reduce(
                    dBf[:, 4096:5120],
                    dBf[:, 0:4096].rearrange("p (dc t) -> p t dc", dc=4),
                    mybir.AxisListType.X, OP.add)
                nc.gpsimd.partition_all_reduce(
                    dBf[:, 5120:6144], dBf[:, 4096:5120], 128, RED.add)
                ACT(dBf[:, 4096:5120], dBf[:, 5120:6144],
                    AF.Abs_reciprocal_sqrt, scale=1.0 / DM, bias=epsc[:])
                TT(mtv(xq[:]), mtv(xT[:]),
                   dBf[:, 4096:5120].unsqueeze(1).broadcast_to([128, 4, 1024]),
                   OP.mult)

                # ================= in_proj xin-half =================
                ps = pp.tile([128, 4096], F32, tag="ps")
                if "noin" in mode:
                    nc.vector.memset(ps[:], 0.0)
                for m in range(0 if "noin" not in mode else 4, 4):
                    for f in range(2):
                        for kt in range(2):
                            off = ((li * 8 + m) * 2 + kt) * 256
                            MM(ps[:, m * 1024 + f * 512: m * 1024 + f * 512 + 512],
                               winT[:, off:off + 256].rearrange("p (i m) -> p i m", i=2),
                               xq[:, 2 * kt * 1024: 2 * (kt + 1) * 1024]
                               .rearrange("p (i t) -> p i t", i=2)[:, :, f * 512:(f + 1) * 512],
                               start=(kt == 0), stop=(kt == 1),
                               perf_mode=mybir.MatmulPerfMode.DoubleRow)
                # re-zero conv left pads (sz overlay clobbered them last layer)
                nc.vector.memset(
                    xinpad[:].rearrange("p (m c) -> p m c", m=4)[:, :, 0:3], 0.0)
                TSM(xinpad[:].rearrange("p (m c) -> p m c", m=4)[:, :, 3:1027],
                    mtv(ps[:]), 1.0 / 64.0)

                # ================= in_proj z-half =================
                psz = pp.tile([128, 4096], F32, tag="ps")
                if "noin" in mode:
                    nc.vector.memset(psz[:], 0.0)
                for m in range(0 if "noin" not in mode else 4, 4):
                    for f in range(2):
                        for kt in range(2):
                            off = ((li * 8 + 4 + m) * 2 + kt) * 256
                            MM(psz[:, m * 1024 + f * 512: m * 1024 + f * 512 + 512],
                               winT[:, off:off + 256].rearrange("p (i m) -> p i m", i=2),
                               xq[:, 2 * kt * 1024: 2 * (kt + 1) * 1024]
                               .rearrange("p (i t) -> p i t", i=2)[:, :, f * 512:(f + 1) * 512],
                               start=(kt == 0), stop=(kt == 1),
                               perf_mode=mybir.MatmulPerfMode.DoubleRow)

                # ================= conv (4 taps) + silu =================
                xpv = xinpad[:].rearrange("p (m c) -> p m c", m=4)
                cw = lambda tap: bc_t(cvw[:, li * 16 + tap * 4: li * 16 + tap * 4 + 4])
                tmpv = mtv(dBxh[:].bitcast(BF)[:, 0:4096])
                TT(mtv(xn[:]), xpv[:, :, 3:1027], cw(3), OP.mult)
                for tap in (2, 1, 0):
                    TT(tmpv, xpv[:, :, tap:tap + 1024], cw(tap), OP.mult)
                    TT(xn[:], xn[:], dBxh[:].bitcast(BF)[:, 0:4096], OP.add)
                if has_cvb:
                    TT(mtv(xn[:]), mtv(xn[:]),
                       bc_t(cvb[:, li * 4: li * 4 + 4]), OP.add)
                ACT(xn[:], xn[:], AF.Silu)   # xn = xin now
                TSM(xinq[:], xn[:], 1.0)

                # silu(z) -> sz (overlay on xinpad[:, 0:4096]; pads re-zeroed next layer)
                ACT(xinpad[:, 0:4096], psz[:], AF.Silu, scale=1.0 / 64.0)

                # ================= x_proj + AllReduce =================
                psx = pp.tile([128, 4096], F32, tag="ps")
                for f in range(2):
                    for kt in range(2):
                        off = (li * 2 + kt) * 128
                        MM(psx[0:64, f * 512: f * 512 + 512],
                           wxp[:, off:off + 128].rearrange("p (i m) -> p i m", i=2),
                           xinq[:, 2 * kt * 1024: 2 * (kt + 1) * 1024]
                           .rearrange("p (i t) -> p i t", i=2)[:, :, f * 512:(f + 1) * 512],
                           start=(kt == 0), stop=(kt == 1),
                           perf_mode=mybir.MatmulPerfMode.DoubleRow)
                TSM(dBxh[:].bitcast(BF)[0:64, 0:1024], psx[0:64, 0:1024], 1.0 / 64.0)
                nc.sync.dma_start(
                    cc[("dbc_in", li)][0:1, :].rearrange("p (a b) -> (p a) b", a=64),
                    dBxh[:].bitcast(BF)[0:64, 0:1024])
                if "nocc" in mode:
                    nc.sync.dma_start(cc[("dbc_out", li)][:], cc[("dbc_in", li)][:])
                else:
                    nc.gpsimd.collective_compute(
                        "AllReduce", OP.add, ins=[cc[("dbc_in", li)][:]],
                        outs=[cc[("dbc_out", li)][:]], replica_groups=RG)
                nc.sync.dma_start(
                    dbcdt[0:32, :],
                    cc[("dbc_out", li)][0:1, 0:32768].rearrange(
                        "p (a b) -> (p a) b", a=32))

                # ================= dt_proj -> delta =================
                psd = pp.tile([128, 4096], F32, tag="ps")
                for m in range(4):
                    for f in range(2):
                        MM(psd[:, m * 1024 + f * 512: m * 1024 + f * 512 + 512],
                           wdt[0:33, li * 512 + m * 128: li * 512 + (m + 1) * 128],
                           dbcdt[0:33, f * 512: f * 512 + 512],
                           start=True, stop=True)
                ACT(dBxh[:].bitcast(BF)[:, 0:4096], psd[:], AF.Exp)
                ACT(lu[:], dBxh[:].bitcast(BF)[:, 0:4096], AF.Ln, bias=1.0)

                TT(ut[:], lu[:], xn[:], OP.mult)
                # ================= B/C broadcast mats =================
                nc.sync.dma_start(BCmat[0:1, :], cc[("dbc_out", li)][0:1, 32768:65536])
                nc.gpsimd.partition_broadcast(BCmat[:], BCmat[0:1, :])

                # ================= selective scan, 4 chunks =================
                psy = pp.tile([128, 4096], F32, tag="ps")
                if "noscan" in mode:
                    nc.vector.memset(psy[:], 0.0)
                kv = kvi[:, li * 16: li * 16 + 16]
                for c in range(0 if "noscan" not in mode else 4, 4):
                    cs = slice(c * 1024, (c + 1) * 1024)
                    # dA-arg (skip boundary col 0 of each n-block)
                    TT(nlv(dA)[:, :, 1:1024],
                       bc_t(kv, 1024)[:, :, 1:1024],
                       lu[:, c * 1024 + 1: (c + 1) * 1024]
                       .unsqueeze(1).broadcast_to([128, N, 1023]),
                       OP.mult)
                    # dA = exp(arg)
                    ACT(nlv(dA)[:, :, 1:1024], nlv(dA)[:, :, 1:1024], AF.Exp)
                    # dBx = u (bcast n) * B
                    TT(nlv(dBxh), bc_n(ut[:, cs]), BCmat[:, 0:16384].rearrange("p (n l) -> p n l", n=N), OP.mult)
                    # h = scan(dA, dBx) in place
                    nc.vector.tensor_tensor_scan(
                        dBxh[:], dA[:], dBxh[:], 0.0, OP.mult, OP.add)
                    # hC = h * C in place
                    TT(dBxh[:], dBxh[:], BCmat[:, 16384:32768], OP.mult)
                    # y_c = sum_n
                    nc.vector.tensor_reduce(
                        psy[:, cs],
                        dBxh[:].rearrange("p (n l) -> p l n", n=N),
                        mybir.AxisListType.X, OP.add)

                # ================= D-term, gate =================
                TT(mtv(dBxh[:].bitcast(BF)[:, 0:4096]), mtv(xn[:]),
                   bc_t(Dv[:, li * 4: li * 4 + 4]), OP.mult)
                TT(dBxh[:].bitcast(BF)[:, 4096:8192], psy[:],
                   dBxh[:].bitcast(BF)[:, 0:4096], OP.add)
                TT(y3q[:], dBxh[:].bitcast(BF)[:, 4096:8192],
                   xinpad[:, 0:4096], OP.mult)   # y3 (x32 via C,D scaling)

                # ================= out_proj + AllReduce + residual =================
                pso = pp.tile([128, 4096], F32, tag="ps")
                if "noout" in mode:
                    nc.vector.memset(pso[:], 0.0)
                for m in range(0 if "noout" not in mode else 4, 4):
                    for f in range(2):
                        for kt in range(2):
                            off = ((li * 4 + m) * 2 + kt) * 256
                            MM(pso[:, m * 1024 + f * 512: m * 1024 + f * 512 + 512],
                               wout[:, off:off + 256].rearrange("p (i m) -> p i m", i=2),
                               y3q[:, 2 * kt * 1024: 2 * (kt + 1) * 1024]
                               .rearrange("p (i t) -> p i t", i=2)[:, :, f * 512:(f + 1) * 512],
                               start=(kt == 0), stop=(kt == 1),
                               perf_mode=mybir.MatmulPerfMode.DoubleRow)
                boq = lu[:].bitcast(E4)[:, 0:4096]
                TSM(boq, pso[:], 1.0 / 32.0)   # 64x-scaled fp8 bo partial
                nc.sync.dma_start(cc[("bo_in", li)][:], boq)
                if "nocc" in mode:
                    nc.sync.dma_start(cc[("bo_out", li)][:], cc[("bo_in", li)][:])
                else:
                    nc.gpsimd.collective_compute(
                        "AllReduce", OP.add, ins=[cc[("bo_in", li)][:]],
                        outs=[cc[("bo_out", li)][:]], replica_groups=RG)
                nc.sync.dma_start(lu[:].bitcast(E4)[:, 0:4096], cc[("bo_out", li)][:])
                nc.vector.scalar_tensor_tensor(
                    xT[:], lu[:].bitcast(E4)[:, 0:4096], 1.0 / 64.0, xT[:],
                    OP.mult, OP.add)

            # ================= head =================
            TT(mtv(dBf[:, 0:4096]), mtv(xT[:]), bc_t(fcp[:]), OP.mult)
            nc.vector.tensor_reduce(
                dBf[:, 4096:5120],
                dBf[:, 0:4096].rearrange("p (dc t) -> p t dc", dc=4),
                mybir.AxisListType.X, OP.add)
            nc.gpsimd.partition_all_reduce(
                dBf[:, 5120:6144], dBf[:, 4096:5120], 128, RED.add)
            ACT(dBf[0:1, 6144:7168], dBf[0:1, 5120:6144],
                AF.Sigmoid, bias=fcb[:])
            nc.sync.dma_start(out_d[:], dBf[0:1, 6144:7168])

    nc.finalize()
    return nc


def _pack_core(inp, b, e):
    sl = slice(e * EDH, (e + 1) * EDH)
    m = {}
    xt = np.asarray(inp["x"])[b].T.astype(np.float32)  # [512, 1024]
    m["xT"] = np.ascontiguousarray(
        xt.reshape(4, 128, 1024).transpose(1, 0, 2).reshape(128, 4096)).astype(BF16)

    winT = np.zeros((128, 8192), F8)
    for li in range(NL):
        Wc = (np.asarray(inp["in_proj_w"])[li].astype(np.float32)
              * np.asarray(inp["norm_w"])[li][None, :].astype(np.float32))
        Wl = np.concatenate([Wc[sl], Wc[ED + e * EDH: ED + (e + 1) * EDH]], 0)  # [1024, 512]
        arr = (Wl * 64.0).reshape(8, 128, 4, 128).transpose(3, 0, 2, 1).reshape(128, 4096)
        winT[:, li * 4096:(li + 1) * 4096] = arr.astype(F8)
    m["winT"] = winT

    wout = np.zeros((128, 4096), F8)
    for li in range(NL):
        Wol = np.asarray(inp["out_proj_w"])[li][:, sl].astype(np.float32)  # [512, 512]
        arr = (Wol * 64.0).reshape(4, 128, 4, 128).transpose(3, 0, 2, 1).reshape(128, 2048)
        wout[:, li * 2048:(li + 1) * 2048] = arr.astype(F8)
    m["wout"] = wout

    wxp = np.zeros((128, 512), F8)
    for li in range(NL):
        Wxl = np.asarray(inp["x_proj_w"])[li][:, sl].astype(np.float32).copy()  # [64, 512]
        Wxl[48:64] *= 32.0   # C rows pre-scaled (y3 fp8 range); undone at out_proj evac
        arr = (Wxl * 64.0).T.reshape(4, 128, 64).transpose(1, 0, 2).reshape(128, 256)
        wxp[:, li * 256:(li + 1) * 256] = arr.astype(F8)
    m["wxp"] = wxp

    wdt = np.zeros((33, 1024), BF16)
    for li in range(NL):
        Wdl = np.asarray(inp["dt_w"])[li][sl].astype(np.float32)  # [512, 32]
        wdt[0:32, li * 512:(li + 1) * 512] = Wdl.T.astype(BF16)
        wdt[32, li * 512:(li + 1) * 512] = \
            np.asarray(inp["dt_b"])[li][sl].astype(np.float32).astype(BF16)
    m["wdt"] = wdt

    cvw = np.zeros((128, 32), np.float32)
    for li in range(NL):
        cw = np.asarray(inp["conv_w"])[li][sl, 0, :].astype(np.float32)  # [512, 4]
        cvw[:, li * 16:(li + 1) * 16] = \
            cw.reshape(4, 128, 4).transpose(1, 2, 0).reshape(128, 16)
    m["cvw"] = cvw

    def cols8(v):
        out = np.zeros((128, 8), np.float32)
        for li in range(NL):
            out[:, li * 4:(li + 1) * 4] = np.asarray(v)[li][sl].astype(
                np.float32).reshape(4, 128).T
        return out

    m["cvb"] = cols8(inp["conv_b"])
    m["Dv"] = cols8(inp["D"]) * 32.0

    kvi = np.zeros((128, 32), np.float32)
    for li in range(NL):
        A = -np.exp(np.asarray(inp["A_log"])[li].astype(np.float64))  # [ED, N]
        kvi[:, li * 16:(li + 1) * 16] = A[0].astype(np.float32)[None, :]
    m["kvi"] = kvi

    fcp = np.zeros((128, 4), np.float32)
    fw = np.asarray(inp["fc_w"]).reshape(-1).astype(np.float32)
    for dc in range(4):
        fcp[:, dc] = fw[dc * 128:(dc + 1) * 128]
    m["fcp"] = fcp
    m["fcb"] = np.array([[float(np.asarray(inp["fc_b"]).reshape(-1)[0])]], np.float32)
    return m


def kernel(**inputs):
    global LAST_RUN_S
    kvals = []
    for li in range(NL):
        A = -np.exp(np.asarray(inputs["A_log"])[li].astype(np.float64))
        a0 = A[0]
        assert np.abs(A - a0[None, :]).max() <= 1e-6 * np.abs(a0).max(), \
            "A not uniform across channels"
        kvals.append(tuple(float(v) for v in a0))
    has_cvb = bool(np.abs(np.asarray(inputs["conv_b"])).max() > 0)
    key = (REPEAT, ABLATE, has_cvb, tuple(kvals))
    if key not in _CACHE:
        _CACHE[key] = _build(REPEAT, kvals, has_cvb, ABLATE)
    nc = _CACHE[key]
    in_maps = [_pack_core(inputs, core // 2, core % 2) for core in range(8)]
    t0 = time.time()
    res = run_bass_kernel_spmd(nc, in_maps, list(range(8)))
    LAST_RUN_S = time.time() - t0
    out = np.concatenate([
        np.asarray(res.results[2 * b]["out"], np.float32).reshape(-1)
        for b in range(B)])
    return out
